# revision 5
# baseline (speedup 1.0000x reference)
"""AspectAttention Trainium2 kernel V4 (8 NeuronCores, pure data parallel).

out[b, n] = sum_e softmax_n(tanh(h @ W_a + b_a))[b, n, e] * h[b, n, e]

Self-contained: hardcodes shapes B=4096, N=64, D=256, 8 cores.

Per-core dataflow (512 batches in 4 chunks of 128, batch-on-partitions):
  - DMA cast-load h chunk -> bf16 SBUF [128 b, 64 n, 256 d]
  - PE transposes h tiles -> PSUM psT [d, b] per (4n, dh)
  - ACT drains psT -> SBUF hT via f32-bitcast views (half the cycles)
  - PE gemm per 8n-group: stationary hT[d, b], moving W[d, e] -> psS[b, 8n, 256]
  - ACT custom LUT: Exp slot evaluates exp(tanh(x)): psS -> E bf16
  - PE S-accumulate: regular matmuls (lhsT=E-tile, rhs=ident) accumulate
    S^T[e, b] = sum_n E in f32 PSUM (exact)
  - DVE reciprocal from PSUM -> R^T; PE transposes R^T back -> R[b, e]
  - DVE: P1 = E*h (flat AP, in-place), Q = P1*R (bcast over n, in-place)
  - GPSIMD(Pool): level-1 e-fold T1 = Q[:,:,0:128]+Q[:,:,128:256]
  - DVE tree: T1 -> out[b, 64] f32; DMA out
The per-chunk R/P1/Q/reduce tail is emitted one chunk late, interleaved
into the next chunk's group loop, so no engine queue head-blocks.
"""
import hashlib
import json
import os
import shutil
from contextlib import ExitStack

import numpy as np

_PWP_SRC = (
    "/nix/store/z022hj2nvbm3nwdizlisq4ylc0y7rd6q-python3-3.13.14-env/"
    "lib/python3.13/site-packages/neuronxcc/pwp/pwp_bin_trainium"
)


def _exptanh_derivs(x):
    u = np.tanh(x)
    s = 1.0 - u * u
    f = np.exp(u)
    return (f, f * s, f * (s * s - 2 * u * s),
            f * (s**3 - 6 * u * s * s - 2 * s * s + 4 * u * u * s))


def _install_act_tables():
    """Build ACT tables where func `exp` evaluates exp(tanh(x)). Returns a
    content hash so the compile cache keys on the table contents."""
    global _PWP_SRC
    if not os.path.isdir(_PWP_SRC):
        from neuronxcc.driver.Job import Job
        from neuronxcc.driver.jobs.support.FindActInfo import findActInfoFile
        _PWP_SRC = os.path.dirname(findActInfoFile(Job.getPackageDir(), "gen3"))
    dst = "/tmp/aspect_act_tables_v1"
    if not os.path.exists(os.path.join(dst, "act_info.json")):
        tmp = dst + ".tmp"
        if os.path.exists(tmp):
            shutil.rmtree(tmp)
        shutil.copytree(_PWP_SRC, tmp)
        bkt_path = os.path.join(tmp, "exp_and_others_bkt.bin")
        b = np.fromfile(bkt_path, dtype=np.float32).reshape(-1, 8).copy()
        x0 = b[:, 4].astype(np.float64)
        d0, d1, d2 = b[:, 0], b[:, 1], b[:, 2]
        with np.errstate(over="ignore", invalid="ignore"):
            ex = np.exp(np.clip(x0, -87.0, 87.0))
            is_exp = (np.isfinite(d0)
                      & (np.abs(d0 - ex) <= 1e-3 * np.maximum(ex, 1e-30))
                      & (np.abs(d1 - d0) <= 1e-3 * np.abs(d0) + 1e-30)
                      & (np.abs(d2 - d0 / 2) <= 1e-3 * np.abs(d0) + 1e-30))
        idx = np.where(is_exp)[0]
        f, f1, f2, f3 = _exptanh_derivs(x0[idx])
        b[idx, 0] = f.astype(np.float32)
        b[idx, 1] = f1.astype(np.float32)
        b[idx, 2] = (f2 / 2.0).astype(np.float32)
        b[idx, 3] = (f3 / 6.0).astype(np.float32)
        b[779] = [np.float32(np.e), 0, 0, 0, 0, 0, 0, 0]
        b[780] = [np.float32(1 / np.e), 0, 0, 0, 0, 0, 0, 0]
        b.tofile(bkt_path)
        pj_path = os.path.join(tmp, "exp_and_others.json")
        pj = json.load(open(pj_path))
        for fm in pj["profile_meta_data"]:
            if fm["func_name"].startswith("exp"):
                fm["fpinf_result"] = int(np.float32(np.e).view(np.uint32))
                fm["fninf_result"] = int(np.float32(1 / np.e).view(np.uint32))
        json.dump(pj, open(pj_path, "w"))
        os.replace(tmp, dst) if not os.path.exists(dst) else None
    os.environ["BASS_ACT_ROOT_JSON_PATH"] = os.path.join(dst, "act_info.json")
    hsh = hashlib.sha256(
        open(os.path.join(dst, "exp_and_others_bkt.bin"), "rb").read()
    ).hexdigest()[:8]
    return hsh

import concourse.bass as bass
import concourse.tile as tile
from concourse import bacc, mybir
from concourse.bass_utils import run_bass_kernel_spmd

N_CORES = 8
B_FULL, N_BLOCK, D = 4096, 64, 256
B_SHARD = B_FULL // N_CORES  # 512
P = 128
N_CHUNKS = B_SHARD // P  # 4
F32 = mybir.dt.float32
BF16 = mybir.dt.bfloat16
ALU = mybir.AluOpType
ACT_T = mybir.ActivationFunctionType


def build_nc(with_bias: bool):
    tbl_hash = _install_act_tables()
    nc = bacc.Bacc("TRN2", debug=False, num_devices=N_CORES)
    tbl_d = nc.dram_tensor(f"tblkey_{tbl_hash}", [1, 4], F32, kind="ExternalInput")
    h_d = nc.dram_tensor("h", [B_SHARD, N_BLOCK, D], F32, kind="ExternalInput")
    w_d = nc.dram_tensor("W_a", [D, D], F32, kind="ExternalInput")
    ident_d = nc.dram_tensor("ident", [P, P], BF16, kind="ExternalInput")
    if with_bias:
        ba_d = nc.dram_tensor("b_a", [N_BLOCK, D], F32, kind="ExternalInput")
    out_d = nc.dram_tensor("out", [B_SHARD, N_BLOCK], F32, kind="ExternalOutput")

    with tile.TileContext(nc) as tc, ExitStack() as ctx:
        const_p = ctx.enter_context(tc.tile_pool(name="const", bufs=1))
        h_p = ctx.enter_context(tc.tile_pool(name="h", bufs=2))
        e_p = ctx.enter_context(tc.tile_pool(name="E", bufs=2))
        ht_p = ctx.enter_context(tc.tile_pool(name="hT", bufs=2))
        scr_p = ctx.enter_context(tc.tile_pool(name="scr", bufs=1))
        r_p = ctx.enter_context(tc.tile_pool(name="R", bufs=1))
        o_p = ctx.enter_context(tc.tile_pool(name="o", bufs=2))
        bb_p = (ctx.enter_context(tc.tile_pool(name="bb", bufs=2))
                if with_bias else None)
        psS_p = ctx.enter_context(tc.tile_pool(name="psS", bufs=1, space="PSUM"))
        psR_p = ctx.enter_context(tc.tile_pool(name="psR", bufs=1, space="PSUM"))

        tblk = const_p.tile([1, 4], F32)
        nc.sync.dma_start(tblk[:], tbl_d.ap())
        ident = const_p.tile([P, P], BF16)
        nc.sync.dma_start(ident[:], ident_d.ap())
        wb = const_p.tile([P, 2, D], BF16)
        nc.gpsimd.dma_start(wb[:, 0, :], w_d.ap()[0:P, :])
        nc.gpsimd.dma_start(wb[:, 1, :], w_d.ap()[P: 2 * P, :])

        state = {}  # per-chunk tiles for the deferred tail

        def emit_loads(c):
            """DMA cast-loads for chunk c + batched DMA-transposes -> hT."""
            bs = c * P
            h_nat = h_p.tile([P, N_BLOCK, D], BF16, name=f"h_nat{c}", tag="hn")
            hT = ht_p.tile([P, N_BLOCK, 2, P], BF16, name=f"hT{c}", tag="ht")
            hTm = hT[:].rearrange("p n h b -> p (n h) b")
            hv = h_nat[:].rearrange("p n d -> p (n d)")
            for g in range(4):
                nc.gpsimd.dma_start(
                    h_nat[:, g * 16: (g + 1) * 16, :],
                    h_d.ap()[bs: bs + P, g * 16: (g + 1) * 16, :],
                )
                nc.sync.dma_start_transpose(
                    hTm[:, g * 32: (g + 1) * 32, :],
                    hv[:, g * 4096: (g + 1) * 4096],
                )
            return h_nat, hT

        def emit_tree(c, E):
            """s4 onwards of the reduction tree + output DMA for chunk c."""
            bs = c * P
            s4 = scr_p.tile([P, N_BLOCK, 16], F32, tag="s4")
            nc.vector.tensor_tensor(s4[:], E[:, :, 0:16], E[:, :, 16:32],
                                    ALU.add)
            s5 = scr_p.tile([P, N_BLOCK, 8], F32, tag="s5")
            nc.vector.tensor_tensor(s5[:], s4[:, :, 0:8], s4[:, :, 8:16],
                                    ALU.add)
            s6 = scr_p.tile([P, N_BLOCK, 4], F32, tag="s6")
            nc.vector.tensor_tensor(s6[:], s5[:, :, 0:4], s5[:, :, 4:8],
                                    ALU.add)
            s7 = scr_p.tile([P, N_BLOCK, 2], F32, tag="s7")
            nc.vector.tensor_tensor(s7[:], s6[:, :, 0:2], s6[:, :, 2:4],
                                    ALU.add)
            out_sb = o_p.tile([P, N_BLOCK], F32, tag="out_sb")
            nc.vector.tensor_tensor(
                out_sb[:].unsqueeze(2), s7[:, :, 0:1], s7[:, :, 1:2], ALU.add)
            nc.gpsimd.dma_start(out_d.ap()[bs: bs + P, :], out_sb[:])

        def emit_tail(c):
            """R/Q/reduce chain for chunk c (emitted during chunk c+1).
            P1 = E*h already happened in-chunk; E holds E*h here."""
            E = state[c]["E"]
            psSS = state[c]["psSS"]
            rT = r_p.tile([P, 2, P], F32, tag="rT")
            nc.vector.reciprocal(rT[:], psSS[:])
            rTb = r_p.tile([P, 2, P], BF16, tag="rTb")
            nc.scalar.copy(rTb[:], rT[:])
            psR = psR_p.tile([P, 2, P], F32, tag="psR")
            for eh in range(2):
                nc.tensor.matmul(psR[:, eh, :], rTb[:, eh, :], ident[:],
                                 start=True, stop=True)
            Rb = r_p.tile([P, D], BF16, tag="Rb")
            nc.scalar.copy(Rb[:].rearrange("p (a b) -> p a b", a=2), psR[:])
            rb_bcast = Rb[:].unsqueeze(1).broadcast_to((P, N_BLOCK, D))
            nc.vector.tensor_tensor(E[:], E[:], rb_bcast, ALU.mult)
            # in-place reduction tree over e inside E
            nc.vector.tensor_tensor(E[:, :, 0:P], E[:, :, 0:P], E[:, :, P:D],
                                    ALU.add)
            nc.vector.tensor_tensor(E[:, :, 0:64], E[:, :, 0:64],
                                    E[:, :, 64:P], ALU.add)
            nc.vector.tensor_tensor(E[:, :, 0:32], E[:, :, 0:32],
                                    E[:, :, 32:64], ALU.add)
            emit_tree(c, E)

        h_next = emit_loads(0)
        for c in range(N_CHUNKS):
            h_nat, hT = h_next
            E = e_p.tile([P, N_BLOCK, D], BF16)
            psSS = psR_p.tile([P, 2, P], F32, tag="psSS")
            state[c] = {"E": E, "h": h_nat, "psSS": psSS}

            for G in range(8):  # 8n-groups
                psS = psS_p.tile([P, 8, D], F32)
                nG = G * 8
                for j in range(8):
                    nc.tensor.matmul(
                        psS[:, j, :], hT[:, nG + j, 0, :], wb[:, 0, :],
                        start=True, stop=False,
                    )
                    nc.tensor.matmul(
                        psS[:, j, :], hT[:, nG + j, 1, :], wb[:, 1, :],
                        start=False, stop=True,
                    )
                if with_bias:
                    bab = bb_p.tile([P, 8, D], BF16)
                    src = ba_d.ap()[nG: nG + 8, :]
                    src = src.rearrange("(one n) d -> one n d", one=1)
                    src = src.broadcast_to((P, 8, D))
                    nc.gpsimd.dma_start(bab[:], src)
                    nc.vector.tensor_add(psS[:], psS[:], bab[:])
                # custom LUT: Exp evaluates exp(tanh(x))
                nc.scalar.activation(E[:, nG: nG + 8, :], psS[:], ACT_T.Exp)
                # S^T accumulation for the PREVIOUS group (keeps PE busy
                # while ACT computes this group's exp)
                if G > 0:
                    nGp = (G - 1) * 8
                    for j in range(8):
                        for eh in range(2):
                            nc.tensor.matmul(
                                psSS[:, eh, :],
                                E[:, nGp + j, eh * P: (eh + 1) * P],
                                ident[:],
                                start=(nGp + j == 0), stop=False,
                            )
                # issue next chunk's loads early so the DMA queue never
                # sits behind the Pool-engine level-1 fold
                if G == 0 and c + 1 < N_CHUNKS:
                    h_next = emit_loads(c + 1)
                # deferred tail of the previous chunk, after this chunk's
                # pipeline is warmed up
                if G == 1 and c > 0:
                    emit_tail(c - 1)
                # in-chunk P1 = E*h for finished group pairs (frees h early
                # and spreads DVE work); S-accum of those n is already done
                if G >= 2 and G % 2 == 0:
                    k = (G - 2) // 2  # quarter 0..2 at G=2,4,6
                    nA = k * 16
                    ef = E[:, nA: nA + 16, :].rearrange("p n d -> p (n d)")
                    hf = h_nat[:, nA: nA + 16, :].rearrange("p n d -> p (n d)")
                    nc.vector.tensor_tensor(ef, ef, hf, ALU.mult)
            nGp = 7 * 8
            for j in range(8):
                for eh in range(2):
                    nc.tensor.matmul(
                        psSS[:, eh, :],
                        E[:, nGp + j, eh * P: (eh + 1) * P],
                        ident[:],
                        start=False, stop=(nGp + j == N_BLOCK - 1),
                    )
            # last quarter of P1 (n 48..63) after the final S-accums
            ef = E[:, 48:64, :].rearrange("p n d -> p (n d)")
            hf = h_nat[:, 48:64, :].rearrange("p n d -> p (n d)")
            nc.vector.tensor_tensor(ef, ef, hf, ALU.mult)
        emit_tail(N_CHUNKS - 1)
    nc.compile()
    return nc


_CACHE = {}


def _get_nc(with_bias: bool):
    if with_bias not in _CACHE:
        _CACHE[with_bias] = build_nc(with_bias)
    return _CACHE[with_bias]


def run(h, W_a, b_a, trace=False):
    import ml_dtypes

    tbl_hash = _install_act_tables()

    h = np.ascontiguousarray(np.asarray(h, dtype=np.float32))
    W_a = np.ascontiguousarray(np.asarray(W_a, dtype=np.float32))
    b_a = np.ascontiguousarray(np.asarray(b_a, dtype=np.float32))
    with_bias = bool(np.any(b_a))
    nc = _get_nc(with_bias)
    ident = np.eye(P, dtype=ml_dtypes.bfloat16)
    in_maps = []
    for i in range(N_CORES):
        m = {
            "h": h[i * B_SHARD: (i + 1) * B_SHARD],
            "W_a": W_a,
            "ident": ident,
            f"tblkey_{tbl_hash}": np.zeros((1, 4), np.float32),
        }
        if with_bias:
            m["b_a"] = b_a
        in_maps.append(m)
    res = run_bass_kernel_spmd(nc, in_maps, core_ids=list(range(N_CORES)), trace=trace)
    out = np.concatenate([res.results[i]["out"] for i in range(N_CORES)], axis=0)
    return out, res


def kernel(h, W_a, b_a):
    out, _ = run(h, W_a, b_a, trace=False)
    return out


# revision 6
# speedup vs baseline: 1.5846x; 1.5846x over previous
"""AspectAttention Trainium2 kernel V4 (8 NeuronCores, pure data parallel).

out[b, n] = sum_e softmax_n(tanh(h @ W_a + b_a))[b, n, e] * h[b, n, e]

Self-contained: hardcodes shapes B=4096, N=64, D=256, 8 cores.

Per-core dataflow (512 batches in 4 chunks of 128, batch-on-partitions):
  - DMA cast-load h chunk -> bf16 SBUF [128 b, 64 n, 256 d]
  - PE transposes h tiles -> PSUM psT [d, b] per (4n, dh)
  - ACT drains psT -> SBUF hT via f32-bitcast views (half the cycles)
  - PE gemm per 8n-group: stationary hT[d, b], moving W[d, e] -> psS[b, 8n, 256]
  - ACT custom LUT: Exp slot evaluates exp(tanh(x)): psS -> E bf16
  - PE S-accumulate: regular matmuls (lhsT=E-tile, rhs=ident) accumulate
    S^T[e, b] = sum_n E in f32 PSUM (exact)
  - DVE reciprocal from PSUM -> R^T; PE transposes R^T back -> R[b, e]
  - DVE: P1 = E*h (flat AP, in-place), Q = P1*R (bcast over n, in-place)
  - GPSIMD(Pool): level-1 e-fold T1 = Q[:,:,0:128]+Q[:,:,128:256]
  - DVE tree: T1 -> out[b, 64] f32; DMA out
The per-chunk R/P1/Q/reduce tail is emitted one chunk late, interleaved
into the next chunk's group loop, so no engine queue head-blocks.
"""
import hashlib
import json
import os
import shutil
from contextlib import ExitStack

import numpy as np

_PWP_SRC = (
    "/nix/store/z022hj2nvbm3nwdizlisq4ylc0y7rd6q-python3-3.13.14-env/"
    "lib/python3.13/site-packages/neuronxcc/pwp/pwp_bin_trainium"
)


def _exptanh_derivs(x):
    u = np.tanh(x)
    s = 1.0 - u * u
    f = np.exp(u)
    return (f, f * s, f * (s * s - 2 * u * s),
            f * (s**3 - 6 * u * s * s - 2 * s * s + 4 * u * u * s))


def _install_act_tables():
    """Build ACT tables where func `exp` evaluates exp(tanh(x)). Returns a
    content hash so the compile cache keys on the table contents."""
    global _PWP_SRC
    if not os.path.isdir(_PWP_SRC):
        from neuronxcc.driver.Job import Job
        from neuronxcc.driver.jobs.support.FindActInfo import findActInfoFile
        _PWP_SRC = os.path.dirname(findActInfoFile(Job.getPackageDir(), "gen3"))
    dst = "/tmp/aspect_act_tables_v1"
    if not os.path.exists(os.path.join(dst, "act_info.json")):
        tmp = dst + ".tmp"
        if os.path.exists(tmp):
            shutil.rmtree(tmp)
        shutil.copytree(_PWP_SRC, tmp)
        bkt_path = os.path.join(tmp, "exp_and_others_bkt.bin")
        b = np.fromfile(bkt_path, dtype=np.float32).reshape(-1, 8).copy()
        x0 = b[:, 4].astype(np.float64)
        d0, d1, d2 = b[:, 0], b[:, 1], b[:, 2]
        with np.errstate(over="ignore", invalid="ignore"):
            ex = np.exp(np.clip(x0, -87.0, 87.0))
            is_exp = (np.isfinite(d0)
                      & (np.abs(d0 - ex) <= 1e-3 * np.maximum(ex, 1e-30))
                      & (np.abs(d1 - d0) <= 1e-3 * np.abs(d0) + 1e-30)
                      & (np.abs(d2 - d0 / 2) <= 1e-3 * np.abs(d0) + 1e-30))
        idx = np.where(is_exp)[0]
        f, f1, f2, f3 = _exptanh_derivs(x0[idx])
        b[idx, 0] = f.astype(np.float32)
        b[idx, 1] = f1.astype(np.float32)
        b[idx, 2] = (f2 / 2.0).astype(np.float32)
        b[idx, 3] = (f3 / 6.0).astype(np.float32)
        b[779] = [np.float32(np.e), 0, 0, 0, 0, 0, 0, 0]
        b[780] = [np.float32(1 / np.e), 0, 0, 0, 0, 0, 0, 0]
        b.tofile(bkt_path)
        pj_path = os.path.join(tmp, "exp_and_others.json")
        pj = json.load(open(pj_path))
        for fm in pj["profile_meta_data"]:
            if fm["func_name"].startswith("exp"):
                fm["fpinf_result"] = int(np.float32(np.e).view(np.uint32))
                fm["fninf_result"] = int(np.float32(1 / np.e).view(np.uint32))
        json.dump(pj, open(pj_path, "w"))
        os.replace(tmp, dst) if not os.path.exists(dst) else None
    os.environ["BASS_ACT_ROOT_JSON_PATH"] = os.path.join(dst, "act_info.json")
    hsh = hashlib.sha256(
        open(os.path.join(dst, "exp_and_others_bkt.bin"), "rb").read()
    ).hexdigest()[:8]
    return hsh

import concourse.bass as bass
import concourse.tile as tile
from concourse import bacc, mybir
from concourse.bass_utils import run_bass_kernel_spmd

N_CORES = 8
B_FULL, N_BLOCK, D = 4096, 64, 256
B_SHARD = B_FULL // N_CORES  # 512
P = 128
N_CHUNKS = B_SHARD // P  # 4
F32 = mybir.dt.float32
BF16 = mybir.dt.bfloat16
ALU = mybir.AluOpType
ACT_T = mybir.ActivationFunctionType


def build_nc(with_bias: bool):
    tbl_hash = _install_act_tables()
    nc = bacc.Bacc("TRN2", debug=False, num_devices=N_CORES)
    tbl_d = nc.dram_tensor(f"tblkey_{tbl_hash}", [1, 4], F32, kind="ExternalInput")
    h_d = nc.dram_tensor("h", [B_SHARD, N_BLOCK, D], F32, kind="ExternalInput")
    w_d = nc.dram_tensor("W_a", [D, D], F32, kind="ExternalInput")
    ident_d = nc.dram_tensor("ident", [P, P], BF16, kind="ExternalInput")
    if with_bias:
        ba_d = nc.dram_tensor("b_a", [N_BLOCK, D], F32, kind="ExternalInput")
    out_d = nc.dram_tensor("out", [B_SHARD, N_BLOCK], F32, kind="ExternalOutput")

    with tile.TileContext(nc) as tc, ExitStack() as ctx:
        const_p = ctx.enter_context(tc.tile_pool(name="const", bufs=1))
        h_p = ctx.enter_context(tc.tile_pool(name="h", bufs=2))
        e_p = ctx.enter_context(tc.tile_pool(name="E", bufs=2))
        ht_p = ctx.enter_context(tc.tile_pool(name="hT", bufs=1))
        t1_p = ctx.enter_context(tc.tile_pool(name="T1", bufs=1))
        scr_p = ctx.enter_context(tc.tile_pool(name="scr", bufs=1))
        r_p = ctx.enter_context(tc.tile_pool(name="R", bufs=2))
        o_p = ctx.enter_context(tc.tile_pool(name="o", bufs=2))
        bb_p = (ctx.enter_context(tc.tile_pool(name="bb", bufs=2))
                if with_bias else None)
        psT_p = ctx.enter_context(tc.tile_pool(name="psT", bufs=2, space="PSUM"))
        psS_p = ctx.enter_context(tc.tile_pool(name="psS", bufs=1, space="PSUM"))
        psR_p = ctx.enter_context(tc.tile_pool(name="psR", bufs=1, space="PSUM"))

        tblk = const_p.tile([1, 4], F32)
        nc.sync.dma_start(tblk[:], tbl_d.ap())
        ident = const_p.tile([P, P], BF16)
        nc.sync.dma_start(ident[:], ident_d.ap())
        wf = const_p.tile([P, 2, D], F32)
        nc.sync.dma_start(wf[:, 0, :], w_d.ap()[0:P, :])
        nc.sync.dma_start(wf[:, 1, :], w_d.ap()[P: 2 * P, :])
        wb = const_p.tile([P, 2, D], BF16)
        nc.vector.tensor_copy(wb[:], wf[:])

        state = {}  # per-chunk tiles for the deferred tail

        def emit_loads(c):
            """DMA cast-loads for chunk c into a fresh h-pool tile."""
            bs = c * P
            h_nat = h_p.tile([P, N_BLOCK, D], BF16, name=f"h_nat{c}", tag="hn")
            npiece = 8 if c == 0 else 4
            step = N_BLOCK // npiece
            for g in range(npiece):
                nc.gpsimd.dma_start(
                    h_nat[:, g * step: (g + 1) * step, :],
                    h_d.ap()[bs: bs + P, g * step: (g + 1) * step, :],
                )
            return h_nat

        def emit_tree(c, s2_in):
            """s2 onwards of the reduction tree + output DMA for chunk c."""
            bs = c * P
            s3 = scr_p.tile([P, N_BLOCK, 32], BF16, tag="s3")
            nc.vector.tensor_tensor(s3[:], s2_in[:, :, 0:32], s2_in[:, :, 32:64],
                                    ALU.add)
            s4 = scr_p.tile([P, N_BLOCK, 16], F32, tag="s4")
            nc.vector.tensor_tensor(s4[:], s3[:, :, 0:16], s3[:, :, 16:32],
                                    ALU.add)
            s5 = scr_p.tile([P, N_BLOCK, 8], F32, tag="s5")
            nc.vector.tensor_tensor(s5[:], s4[:, :, 0:8], s4[:, :, 8:16],
                                    ALU.add)
            s6 = scr_p.tile([P, N_BLOCK, 4], F32, tag="s6")
            nc.vector.tensor_tensor(s6[:], s5[:, :, 0:4], s5[:, :, 4:8],
                                    ALU.add)
            s7 = scr_p.tile([P, N_BLOCK, 2], F32, tag="s7")
            nc.vector.tensor_tensor(s7[:], s6[:, :, 0:2], s6[:, :, 2:4],
                                    ALU.add)
            out_sb = o_p.tile([P, N_BLOCK], F32, tag="out_sb")
            nc.vector.tensor_tensor(
                out_sb[:].unsqueeze(2), s7[:, :, 0:1], s7[:, :, 1:2], ALU.add)
            nc.gpsimd.dma_start(out_d.ap()[bs: bs + P, :], out_sb[:])

        def emit_tail(c):
            """R/Q/reduce chain for chunk c (emitted during chunk c+1).
            P1 = E*h already happened in-chunk; E holds E*h here."""
            E = state[c]["E"]
            psSS = state[c]["psSS"]
            rT = r_p.tile([P, 2, P], F32, tag="rT")
            nc.vector.reciprocal(rT[:], psSS[:])
            rTb = r_p.tile([P, 2, P], BF16, tag="rTb")
            nc.scalar.copy(rTb[:], rT[:])
            psR = psR_p.tile([P, 2, P], F32, tag="psR")
            for eh in range(2):
                nc.tensor.matmul(psR[:, eh, :], rTb[:, eh, :], ident[:],
                                 start=True, stop=True)
            Rb = r_p.tile([P, D], BF16, tag="Rb")
            nc.scalar.copy(Rb[:].rearrange("p (a b) -> p a b", a=2), psR[:])
            # Q = P1*R with a doubled-R broadcast (fewer, longer inner runs)
            Rb2 = r_p.tile([P, 2, D], BF16, tag="Rb2")
            nc.scalar.copy(
                Rb2[:], Rb[:].unsqueeze(1).broadcast_to((P, 2, D)))
            rb_bcast = (Rb2[:].rearrange("p a d -> p (a d)")
                        .unsqueeze(1).broadcast_to((P, N_BLOCK // 2, 2 * D)))
            e_2d = E[:].rearrange("p (m k) d -> p m (k d)", k=2)
            nc.vector.tensor_tensor(e_2d, e_2d, rb_bcast, ALU.mult)
            T1 = t1_p.tile([P, N_BLOCK, P], BF16)
            nc.vector.tensor_tensor(T1[:], E[:, :, 0:P], E[:, :, P:D], ALU.add)
            s2 = scr_p.tile([P, N_BLOCK, 64], BF16, tag="s2")
            nc.vector.tensor_tensor(s2[:], T1[:, :, 0:64], T1[:, :, 64:128],
                                    ALU.add)
            emit_tree(c, s2)

        h_next = emit_loads(0)
        for c in range(N_CHUNKS):
            h_nat = h_next
            E = e_p.tile([P, N_BLOCK, D], BF16)
            hT = ht_p.tile([P, N_BLOCK, 2, P], BF16)  # [dl, n, dh, b]
            hTv = hT[:].bitcast(F32)  # [dl, n, 2, 64]
            psSS = psR_p.tile([P, 2, P], F32, tag="psSS")
            state[c] = {"E": E, "h": h_nat, "psSS": psSS}

            def emit_tc(g4, h_nat=h_nat, hTv=hTv):
                """Transposes + ACT drain copy for one 4n-group."""
                n0 = g4 * 4
                psT = psT_p.tile([P, 4, 2, P], BF16)
                for j in range(4):
                    for dh in range(2):
                        nc.tensor.transpose(
                            psT[:, j, dh, :],
                            h_nat[:, n0 + j, dh * P: (dh + 1) * P],
                            ident[:],
                        )
                nc.scalar.copy(hTv[:, n0: n0 + 4, :, :], psT[:].bitcast(F32))

            # first two 4n-groups ahead of the G-loop; inside the loop each
            # iteration pre-issues the NEXT gemm's transposes+copies so the
            # ACT copies never sit between exp(G-1) and gemm(G) in any queue
            emit_tc(0)
            emit_tc(1)
            for G in range(8):  # 8n-groups
                if G < 7:
                    emit_tc(2 * G + 2)
                    emit_tc(2 * G + 3)
                psS = psS_p.tile([P, 8, D], F32)
                nG = G * 8
                for j in range(8):
                    nc.tensor.matmul(
                        psS[:, j, :], hT[:, nG + j, 0, :], wb[:, 0, :],
                        start=True, stop=False,
                    )
                    nc.tensor.matmul(
                        psS[:, j, :], hT[:, nG + j, 1, :], wb[:, 1, :],
                        start=False, stop=True,
                    )
                if with_bias:
                    bab = bb_p.tile([P, 8, D], BF16)
                    src = ba_d.ap()[nG: nG + 8, :]
                    src = src.rearrange("(one n) d -> one n d", one=1)
                    src = src.broadcast_to((P, 8, D))
                    nc.gpsimd.dma_start(bab[:], src)
                    nc.vector.tensor_add(psS[:], psS[:], bab[:])
                # custom LUT: Exp evaluates exp(tanh(x))
                nc.scalar.activation(E[:, nG: nG + 8, :], psS[:], ACT_T.Exp)
                # S^T accumulation for the PREVIOUS group (keeps PE busy
                # while ACT computes this group's exp)
                if G > 0:
                    nGp = (G - 1) * 8
                    for j in range(8):
                        for eh in range(2):
                            nc.tensor.matmul(
                                psSS[:, eh, :],
                                E[:, nGp + j, eh * P: (eh + 1) * P],
                                ident[:],
                                start=(nGp + j == 0), stop=False,
                            )
                # issue next chunk's loads early so the DMA queue never
                # sits behind the Pool-engine level-1 fold
                if G == 0 and c + 1 < N_CHUNKS:
                    h_next = emit_loads(c + 1)
                # deferred tail of the previous chunk, after this chunk's
                # pipeline is warmed up
                if G == 1 and c > 0:
                    emit_tail(c - 1)
                # in-chunk P1 = E*h for finished group pairs (frees h early
                # and spreads DVE work); S-accum of those n is already done
                if G >= 2 and G % 2 == 0:
                    k = (G - 2) // 2  # quarter 0..2 at G=2,4,6
                    nA = k * 16
                    ef = E[:, nA: nA + 16, :].rearrange("p n d -> p (n d)")
                    hf = h_nat[:, nA: nA + 16, :].rearrange("p n d -> p (n d)")
                    nc.vector.tensor_tensor(ef, ef, hf, ALU.mult)
            nGp = 7 * 8
            for j in range(8):
                for eh in range(2):
                    nc.tensor.matmul(
                        psSS[:, eh, :],
                        E[:, nGp + j, eh * P: (eh + 1) * P],
                        ident[:],
                        start=False, stop=(nGp + j == N_BLOCK - 1),
                    )
            # last quarter of P1 (n 48..63) after the final S-accums
            ef = E[:, 48:64, :].rearrange("p n d -> p (n d)")
            hf = h_nat[:, 48:64, :].rearrange("p n d -> p (n d)")
            nc.vector.tensor_tensor(ef, ef, hf, ALU.mult)
        emit_tail(N_CHUNKS - 1)
    nc.compile()
    return nc


_CACHE = {}


def _get_nc(with_bias: bool):
    if with_bias not in _CACHE:
        _CACHE[with_bias] = build_nc(with_bias)
    return _CACHE[with_bias]


def run(h, W_a, b_a, trace=False):
    import ml_dtypes

    tbl_hash = _install_act_tables()

    h = np.ascontiguousarray(np.asarray(h, dtype=np.float32))
    W_a = np.ascontiguousarray(np.asarray(W_a, dtype=np.float32))
    b_a = np.ascontiguousarray(np.asarray(b_a, dtype=np.float32))
    with_bias = bool(np.any(b_a))
    nc = _get_nc(with_bias)
    ident = np.eye(P, dtype=ml_dtypes.bfloat16)
    in_maps = []
    for i in range(N_CORES):
        m = {
            "h": h[i * B_SHARD: (i + 1) * B_SHARD],
            "W_a": W_a,
            "ident": ident,
            f"tblkey_{tbl_hash}": np.zeros((1, 4), np.float32),
        }
        if with_bias:
            m["b_a"] = b_a
        in_maps.append(m)
    res = run_bass_kernel_spmd(nc, in_maps, core_ids=list(range(N_CORES)), trace=trace)
    out = np.concatenate([res.results[i]["out"] for i in range(N_CORES)], axis=0)
    return out, res


def kernel(h, W_a, b_a):
    out, _ = run(h, W_a, b_a, trace=False)
    return out
